# revision 6
# baseline (speedup 1.0000x reference)
"""Multi-head attention (B=2, S=2048, HID=1024, NH=16, HD=64) with
interleaved RoPE and causal softmax, sharded over 8 TRN2 NeuronCores:
data parallel on batch (cores 0-3 -> b=0, 4-7 -> b=1), tensor parallel
on heads (4 heads per core). Each core computes its heads' partial
o_proj [2048, 1024]; the host sums 4 partials per batch."""
import sys
sys.path.insert(0, "/opt/trn_rl_repo")
import numpy as np

B, S, HID, NH, HD = 2, 2048, 1024, 16, 64
BASE = 10000.0
NCORES = 8
HPC = 4           # heads per core
SC = 512          # free-dim chunk
NSC = S // SC     # 4
NKC = S // 128    # 16 k-chunks of 128
NIC = HID // 128  # 8 input-dim chunks


def _legalize_single_wait(nc):
    """This walrus build rejects instructions with >1 sync wait: split
    extra waits into single-wait Drain instructions on the same engine."""
    import bass_rust
    n_split = 0
    for _name, bbw in nc.bb_map.items():
        bb = getattr(bbw, "bb", bbw)
        out = []
        for inst in bb.instructions:
            si = inst.sync_info
            waits = list(si.on_wait) if si is not None and si.on_wait else []
            if len(waits) > 1:
                for w in waits[:-1]:
                    d = bass_rust.InstDrain(name=nc.get_next_instruction_name())
                    d.engine = inst.engine
                    d.sync_info = bass_rust.SyncInfo(on_wait=[w], on_update=[])
                    nc.register_instruction(d)
                    out.append(d)
                    n_split += 1
                inst.sync_info = bass_rust.SyncInfo(
                    on_wait=[waits[-1]],
                    on_update=list(si.on_update) if si.on_update else [],
                )
            out.append(inst)
        bb.instructions = out
    return n_split


def _build_nc():
    import concourse.bass as bass
    import concourse.tile as tile
    from concourse import mybir

    FP32 = mybir.dt.float32
    FP32R = mybir.dt.float32r
    AF = mybir.ActivationFunctionType
    ALU = mybir.AluOpType

    nc = bass.Bass()
    xq_d = nc.dram_tensor("xqT", (HID, S), FP32R, kind="ExternalInput")
    xk_d = nc.dram_tensor("xkT", (HID, S), FP32R, kind="ExternalInput")
    xv_d = nc.dram_tensor("xvT", (HID, S), FP32R, kind="ExternalInput")
    wq_d = nc.dram_tensor("wqT", (HID, 256), FP32R, kind="ExternalInput")
    wk_d = nc.dram_tensor("wkT", (HID, 256), FP32R, kind="ExternalInput")
    wv_d = nc.dram_tensor("wvT", (HID, 256), FP32R, kind="ExternalInput")
    wo_d = nc.dram_tensor("woT", (256, HID), FP32R, kind="ExternalInput")
    cos_d = nc.dram_tensor("cosT", (128, S), FP32, kind="ExternalInput")
    sin_d = nc.dram_tensor("sinT", (128, S), FP32, kind="ExternalInput")
    r2s_d = nc.dram_tensor("r2s", (128, 128), FP32R, kind="ExternalInput")
    mw_d = nc.dram_tensor("maskw", (128, 896), FP32, kind="ExternalInput")
    out_d = nc.dram_tensor("partial", (S, HID), FP32, kind="ExternalOutput")

    def dma3(dst, src_t, row_stride, nchunk, ncol, offset):
        nc.sync.dma_start(out=dst, in_=bass.AP(
            tensor=src_t, offset=offset,
            ap=[[row_stride, 128], [128 * row_stride, nchunk], [1, ncol]]))

    with tile.TileContext(nc) as tc:
        with tc.tile_pool(name="sbuf", bufs=1) as pool, \
             tc.tile_pool(name="psum", bufs=1, space="PSUM") as pp:
            # persistent tiles
            wq_s = pool.tile((128, NIC, 256), FP32R)
            wk_s = pool.tile((128, NIC, 256), FP32R)
            wv_s = pool.tile((128, NIC, 256), FP32R)
            wo_s = pool.tile((128, 2, HID), FP32R)
            cos_s = pool.tile((128, S), FP32)
            sin_s = pool.tile((128, S), FP32)
            r2s_s = pool.tile((128, 128), FP32R)
            mw_s = pool.tile((128, 896), FP32)
            qr_s = pool.tile((128, 2, S), FP32R)
            kr_s = pool.tile((128, 2, S), FP32R)
            vh1_s = pool.tile((128, NKC, HPC, 128), FP32R)
            ao_s = pool.tile((128, 2, S), FP32R)

            dma3(wq_s, wq_d, 256, NIC, 256, 0)
            dma3(wk_s, wk_d, 256, NIC, 256, 0)
            dma3(wv_s, wv_d, 256, NIC, 256, 0)
            dma3(wo_s, wo_d, HID, 2, HID, 0)
            nc.sync.dma_start(out=cos_s, in_=cos_d[:, :])
            nc.sync.dma_start(out=sin_s, in_=sin_d[:, :])
            nc.sync.dma_start(out=r2s_s, in_=r2s_d[:, :])
            nc.sync.dma_start(out=mw_s, in_=mw_d[:, :])
            nc.vector.memset(vh1_s.bitcast(FP32)[:, :, :, 64:128], 1.0)

            # ---- q/k projections + RoPE -> qr_s/kr_s [128(pair d), 2, S]
            for x_d, w_s, dst in ((xq_d, wq_s, qr_s), (xk_d, wk_s, kr_s)):
                for c4 in range(NSC):
                    x_t = pool.tile((128, NIC, SC), FP32R, name="x_t", bufs=2)
                    dma3(x_t, x_d, S, NIC, SC, SC * c4)
                    for p in range(2):
                        ps_p = pp.tile((128, SC), FP32, name="ps_p", bufs=2)
                        for ck in range(NIC):
                            nc.tensor.matmul(
                                ps_p, w_s[:, ck, 128 * p:128 * p + 128],
                                x_t[:, ck, :],
                                start=(ck == 0), stop=(ck == NIC - 1))
                        qh_t = pool.tile((128, SC), FP32R, name="qh_t", bufs=2)
                        nc.scalar.copy(out=qh_t, in_=ps_p)
                        ps_sw = pp.tile((128, SC), FP32, name="ps_sw", bufs=2)
                        nc.tensor.matmul(ps_sw, r2s_s, qh_t,
                                         start=True, stop=True)
                        dv = dst[:, p, SC * c4:SC * (c4 + 1)]
                        nc.vector.tensor_tensor(
                            out=dv, in0=qh_t,
                            in1=cos_s[:, SC * c4:SC * (c4 + 1)],
                            op=ALU.mult)
                        rt_t = pool.tile((128, SC), FP32, name="rt_t",
                                         bufs=2)
                        nc.vector.tensor_tensor(
                            out=rt_t, in0=ps_sw,
                            in1=sin_s[:, SC * c4:SC * (c4 + 1)],
                            op=ALU.mult)
                        nc.vector.tensor_tensor(
                            out=dv, in0=dv, in1=rt_t, op=ALU.add)

            # ---- v projection -> vh1_s [128(s), kc, h, 0:64] (+ones cols)
            for c4 in range(NSC):
                x_t = pool.tile((128, NIC, SC), FP32R, name="x_t", bufs=2)
                dma3(x_t, xv_d, S, NIC, SC, SC * c4)
                for sub in range(4):
                    ps_v = pp.tile((128, SC), FP32, name="ps_p", bufs=2)
                    for ck in range(NIC):
                        nc.tensor.matmul(
                            ps_v[:, 0:256],
                            x_t[:, ck, 128 * sub:128 * sub + 128],
                            wv_s[:, ck, :],
                            start=(ck == 0), stop=(ck == NIC - 1))
                    nc.vector.tensor_copy(
                        out=vh1_s[:, 4 * c4 + sub, :, 0:64],
                        in_=ps_v[:, 0:256])

            # ---- attention per (head, q-chunk)
            for h in range(HPC):
                p, o = h // 2, 64 * (h % 2)
                for qc in range(NSC):
                    nkc = 4 * (qc + 1)
                    ps_ao = pp.tile((128, SC), FP32, name="ps_ao", bufs=2)
                    p_prev = None
                    for kc in range(nkc):
                        ps_sc = pp.tile((128, SC), FP32, name="ps_sc", bufs=2)
                        nc.tensor.matmul(
                            ps_sc,
                            kr_s[o:o + 64, p, 128 * kc:128 * kc + 128],
                            qr_s[o:o + 64, p, SC * qc:SC * (qc + 1)],
                            start=True, stop=True)
                        p_t = pool.tile((128, SC), FP32R, name="p_t", bufs=3)
                        nc.scalar.activation(out=p_t, in_=ps_sc, func=AF.Exp,
                                             scale=0.125)
                        r = kc - 4 * qc
                        if r >= 0:
                            nc.vector.tensor_tensor(
                                out=p_t, in0=p_t,
                                in1=mw_s[:, 384 - 128 * r:896 - 128 * r],
                                op=ALU.mult)
                        if p_prev is not None:
                            pk, pt = p_prev
                            nc.tensor.matmul(ps_ao, vh1_s[:, pk, h, :], pt,
                                             start=(pk == 0), stop=False)
                        p_prev = (kc, p_t)
                    pk, pt = p_prev
                    nc.tensor.matmul(ps_ao, vh1_s[:, pk, h, :], pt,
                                     start=(pk == 0), stop=True)
                    rc_t = pool.tile((64, SC), FP32, name="rc_t", bufs=2)
                    nc.vector.reciprocal(rc_t, ps_ao[64:128, :])
                    nc.vector.tensor_tensor(
                        out=ao_s[o:o + 64, p, SC * qc:SC * (qc + 1)],
                        in0=ps_ao[0:64, :], in1=rc_t, op=ALU.mult)

            # ---- o_proj partial: [S, HID] = ao.T @ woT
            for sc in range(NKC):
                ot_t = pool.tile((128, HID), FP32, name="ot_t", bufs=2)
                for hc in range(2):
                    ps_o = pp.tile((128, SC), FP32, name="ps_p", bufs=2)
                    for p in range(2):
                        nc.tensor.matmul(
                            ps_o, ao_s[:, p, 128 * sc:128 * sc + 128],
                            wo_s[:, p, SC * hc:SC * (hc + 1)],
                            start=(p == 0), stop=(p == 1))
                    nc.scalar.copy(out=ot_t[:, SC * hc:SC * (hc + 1)],
                                   in_=ps_o)
                nc.sync.dma_start(out=out_d[128 * sc:128 * sc + 128, :],
                                  in_=ot_t)

    _legalize_single_wait(nc)
    return nc


def _host_tables():
    j = np.arange(0, HD, 2, dtype=np.float64)
    theta = BASE ** (-j / HD)                      # [32]
    pos = np.arange(S, dtype=np.float64)
    mt = np.outer(theta, pos)                      # [32, S]
    cosT = np.tile(np.repeat(np.cos(mt), 2, axis=0), (2, 1)).astype(np.float32)
    sinT = np.tile(np.repeat(np.sin(mt), 2, axis=0), (2, 1)).astype(np.float32)
    r2s = np.zeros((128, 128), np.float32)
    i = np.arange(0, 128, 2)
    r2s[i, i + 1] = 1.0    # (R2s.T @ x)[2i+1] = +x[2i]
    r2s[i + 1, i] = -1.0   # (R2s.T @ x)[2i]   = -x[2i+1]
    ii = np.arange(128)[:, None]
    tt = np.arange(896)[None, :]
    mw = (ii <= tt - 384).astype(np.float32)
    return cosT, sinT, r2s, mw


_CACHE = {}


def _get_runner():
    if "runner" in _CACHE:
        return _CACHE["runner"]
    import jax
    from jax.sharding import Mesh, PartitionSpec
    from jax.experimental.shard_map import shard_map
    from concourse import mybir
    import concourse.bass2jax as bass2jax

    nc = _build_nc()
    bass2jax.install_neuronx_cc_hook()

    partition_name = (nc.partition_id_tensor.name
                      if nc.partition_id_tensor else None)
    in_names, out_names, out_avals, zero_shapes = [], [], [], []
    for alloc in nc.m.functions[0].allocations:
        if not isinstance(alloc, mybir.MemoryLocationSet):
            continue
        name = alloc.memorylocations[0].name
        if alloc.kind == "ExternalInput":
            if name != partition_name:
                in_names.append(name)
        elif alloc.kind == "ExternalOutput":
            shape = tuple(alloc.tensor_shape)
            dtype = mybir.dt.np(alloc.dtype)
            out_avals.append(jax.core.ShapedArray(shape, dtype))
            out_names.append(name)
            zero_shapes.append((shape, dtype))
    n_params = len(in_names)
    all_in = list(in_names) + list(out_names)
    if partition_name is not None:
        all_in.append(partition_name)
    donate = tuple(range(n_params, n_params + len(out_names)))

    def _body(*args):
        operands = list(args)
        if partition_name is not None:
            operands.append(bass2jax.partition_id_tensor())
        outs = bass2jax._bass_exec_p.bind(
            *operands, out_avals=tuple(out_avals), in_names=tuple(all_in),
            out_names=tuple(out_names), lowering_input_output_aliases=(),
            sim_require_finite=True, sim_require_nnan=True, nc=nc)
        return tuple(outs)

    devices = jax.devices()[:NCORES]
    mesh = Mesh(np.asarray(devices), ("core",))
    in_specs = (PartitionSpec("core"),) * (n_params + len(out_names))
    out_specs = (PartitionSpec("core"),) * len(out_names)
    sharded = jax.jit(
        shard_map(_body, mesh=mesh, in_specs=in_specs, out_specs=out_specs,
                  check_rep=False),
        donate_argnums=donate, keep_unused=True)

    runner = {"sharded": sharded, "in_names": in_names,
              "out_names": out_names, "zero_shapes": zero_shapes}
    _CACHE["runner"] = runner
    return runner


def _make_in_maps(q, k, v, Wq, Wk, Wv, Wo):
    cosT, sinT, r2s, mw = _host_tables()
    qT = [np.ascontiguousarray(q[b].T) for b in range(B)]
    kT = [np.ascontiguousarray(k[b].T) for b in range(B)]
    vT = [np.ascontiguousarray(v[b].T) for b in range(B)]
    WqT, WkT, WvT, WoT = Wq.T, Wk.T, Wv.T, Wo.T
    in_maps = []
    for c in range(NCORES):
        b, g = c // 4, c % 4
        in_maps.append({
            "xqT": qT[b], "xkT": kT[b], "xvT": vT[b],
            "wqT": np.ascontiguousarray(WqT[:, 256 * g:256 * (g + 1)]),
            "wkT": np.ascontiguousarray(WkT[:, 256 * g:256 * (g + 1)]),
            "wvT": np.ascontiguousarray(WvT[:, 256 * g:256 * (g + 1)]),
            "woT": np.ascontiguousarray(WoT[256 * g:256 * (g + 1), :]),
            "cosT": cosT, "sinT": sinT, "r2s": r2s, "maskw": mw,
        })
    return in_maps


def _run(in_maps):
    r = _get_runner()
    per_core = [[np.asarray(m[n]) for n in r["in_names"]] for m in in_maps]
    concat_in = [np.concatenate([per_core[c][i] for c in range(NCORES)],
                                axis=0) for i in range(len(r["in_names"]))]
    concat_zeros = [np.zeros((NCORES * s[0], *s[1:]), d)
                    for s, d in r["zero_shapes"]]
    outs = r["sharded"](*concat_in, *concat_zeros)
    return outs


def kernel(q, k, v, mask, Wq, Wk, Wv, Wo):
    in_maps = _make_in_maps(np.asarray(q), np.asarray(k), np.asarray(v),
                            np.asarray(Wq), np.asarray(Wk), np.asarray(Wv),
                            np.asarray(Wo))
    outs = _run(in_maps)
    part = np.asarray(outs[0]).reshape(NCORES, S, HID)
    out = np.empty((B, S, HID), np.float32)
    out[0] = part[0] + part[1] + part[2] + part[3]
    out[1] = part[4] + part[5] + part[6] + part[7]
    return out


# revision 10
# speedup vs baseline: 64.1875x; 64.1875x over previous
"""Multi-head attention (B=2, S=2048, HID=1024, NH=16, HD=64) with
interleaved RoPE and causal softmax, sharded over 8 TRN2 NeuronCores:
data parallel on batch (cores 0-3 -> b=0, 4-7 -> b=1), tensor parallel
on heads (4 heads per core). Each core computes its heads' partial
o_proj [2048, 1024]; the host sums 4 partials per batch."""
import sys
sys.path.insert(0, "/opt/trn_rl_repo")
import numpy as np

B, S, HID, NH, HD = 2, 2048, 1024, 16, 64
BASE = 10000.0
NCORES = 8
HPC = 4           # heads per core
SC = 512          # free-dim chunk
NSC = S // SC     # 4
NKC = S // 128    # 16 k-chunks of 128
NIC = HID // 128  # 8 input-dim chunks


def _legalize_single_wait(nc):
    """This walrus build rejects instructions with >1 sync wait: split
    extra waits into single-wait Drain instructions on the same engine."""
    import bass_rust
    n_split = 0
    for _name, bbw in nc.bb_map.items():
        bb = getattr(bbw, "bb", bbw)
        out = []
        for inst in bb.instructions:
            si = inst.sync_info
            waits = list(si.on_wait) if si is not None and si.on_wait else []
            if len(waits) > 1:
                for w in waits[:-1]:
                    d = bass_rust.InstDrain(name=nc.get_next_instruction_name())
                    d.engine = inst.engine
                    d.sync_info = bass_rust.SyncInfo(on_wait=[w], on_update=[])
                    nc.register_instruction(d)
                    out.append(d)
                    n_split += 1
                inst.sync_info = bass_rust.SyncInfo(
                    on_wait=[waits[-1]],
                    on_update=list(si.on_update) if si.on_update else [],
                )
            out.append(inst)
        bb.instructions = out
    return n_split


def _build_nc():
    import concourse.bass as bass
    import concourse.tile as tile
    from concourse import mybir

    FP32 = mybir.dt.float32
    FP32R = mybir.dt.float32r
    AF = mybir.ActivationFunctionType
    ALU = mybir.AluOpType

    nc = bass.Bass()
    xq_d = nc.dram_tensor("xqT", (HID, S), FP32R, kind="ExternalInput")
    xk_d = nc.dram_tensor("xkT", (HID, S), FP32R, kind="ExternalInput")
    xv_d = nc.dram_tensor("xvT", (HID, S), FP32R, kind="ExternalInput")
    wq_d = nc.dram_tensor("wqT", (HID, 256), FP32R, kind="ExternalInput")
    wk_d = nc.dram_tensor("wkT", (HID, 256), FP32R, kind="ExternalInput")
    wv_d = nc.dram_tensor("wvT", (HID, 256), FP32R, kind="ExternalInput")
    wo_d = nc.dram_tensor("woT", (256, HID), FP32R, kind="ExternalInput")
    cos_d = nc.dram_tensor("cosT", (128, S), FP32, kind="ExternalInput")
    sin_d = nc.dram_tensor("sinT", (128, S), FP32, kind="ExternalInput")
    r2s_d = nc.dram_tensor("r2s", (128, 128), FP32R, kind="ExternalInput")
    mw_d = nc.dram_tensor("maskw", (128, 896), FP32, kind="ExternalInput")
    out_d = nc.dram_tensor("partial", (S, HID), FP32, kind="ExternalOutput")

    def dma3(dst, src_t, row_stride, nchunk, ncol, offset):
        nc.sync.dma_start(out=dst, in_=bass.AP(
            tensor=src_t, offset=offset,
            ap=[[row_stride, 128], [128 * row_stride, nchunk], [1, ncol]]))

    with tile.TileContext(nc) as tc:
        with tc.tile_pool(name="sbuf", bufs=1) as pool, \
             tc.tile_pool(name="psum", bufs=1, space="PSUM") as pp:
            # persistent tiles
            wq_s = pool.tile((128, NIC, 256), FP32R)
            wk_s = pool.tile((128, NIC, 256), FP32R)
            wv_s = pool.tile((128, NIC, 256), FP32R)
            wo_s = pool.tile((128, 2, HID), FP32R)
            cos_s = pool.tile((128, S), FP32)
            sin_s = pool.tile((128, S), FP32)
            r2s_s = pool.tile((128, 128), FP32R)
            mw_s = pool.tile((128, 896), FP32)
            qr_s = pool.tile((128, 2, S), FP32R)
            kr_s = pool.tile((128, 2, S), FP32R)
            vh1_s = pool.tile((128, NKC, HPC, 128), FP32R)
            ao_s = pool.tile((128, 2, S), FP32R)

            dma3(wq_s, wq_d, 256, NIC, 256, 0)
            dma3(wk_s, wk_d, 256, NIC, 256, 0)
            dma3(wv_s, wv_d, 256, NIC, 256, 0)
            dma3(wo_s, wo_d, HID, 2, HID, 0)
            nc.sync.dma_start(out=cos_s, in_=cos_d[:, :])
            nc.sync.dma_start(out=sin_s, in_=sin_d[:, :])
            nc.sync.dma_start(out=r2s_s, in_=r2s_d[:, :])
            nc.sync.dma_start(out=mw_s, in_=mw_d[:, :])
            nc.vector.memset(vh1_s.bitcast(FP32)[:, :, :, 64:128], 1.0)

            # ---- q/k projections + RoPE -> qr_s/kr_s [128(pair d), 2, S]
            for x_d, w_s, dst in ((xq_d, wq_s, qr_s), (xk_d, wk_s, kr_s)):
                for c4 in range(NSC):
                    x_t = pool.tile((128, NIC, SC), FP32R, name="x_t", bufs=2)
                    dma3(x_t, x_d, S, NIC, SC, SC * c4)
                    for p in range(2):
                        ps_p = pp.tile((128, SC), FP32, name="ps_p", bufs=2)
                        for ck in range(NIC):
                            nc.tensor.matmul(
                                ps_p, w_s[:, ck, 128 * p:128 * p + 128],
                                x_t[:, ck, :],
                                start=(ck == 0), stop=(ck == NIC - 1))
                        qh_t = pool.tile((128, SC), FP32R, name="qh_t", bufs=2)
                        nc.scalar.copy(out=qh_t, in_=ps_p)
                        ps_sw = pp.tile((128, SC), FP32, name="ps_sw", bufs=2)
                        nc.tensor.matmul(ps_sw, r2s_s, qh_t,
                                         start=True, stop=True)
                        dv = dst[:, p, SC * c4:SC * (c4 + 1)]
                        nc.vector.tensor_tensor(
                            out=dv, in0=qh_t,
                            in1=cos_s[:, SC * c4:SC * (c4 + 1)],
                            op=ALU.mult)
                        rt_t = pool.tile((128, SC), FP32, name="rt_t",
                                         bufs=2)
                        nc.vector.tensor_tensor(
                            out=rt_t, in0=ps_sw,
                            in1=sin_s[:, SC * c4:SC * (c4 + 1)],
                            op=ALU.mult)
                        nc.vector.tensor_tensor(
                            out=dv, in0=dv, in1=rt_t, op=ALU.add)

            # ---- v projection -> vh1_s [128(s), kc, h, 0:64] (+ones cols)
            for c4 in range(NSC):
                x_t = pool.tile((128, NIC, SC), FP32R, name="x_t", bufs=2)
                dma3(x_t, xv_d, S, NIC, SC, SC * c4)
                for sub in range(4):
                    ps_v = pp.tile((128, SC), FP32, name="ps_p", bufs=2)
                    for ck in range(NIC):
                        nc.tensor.matmul(
                            ps_v[:, 0:256],
                            x_t[:, ck, 128 * sub:128 * sub + 128],
                            wv_s[:, ck, :],
                            start=(ck == 0), stop=(ck == NIC - 1))
                    nc.vector.tensor_copy(
                        out=vh1_s[:, 4 * c4 + sub, :, 0:64],
                        in_=ps_v[:, 0:256])

            # ---- attention per (head, q-chunk)
            for h in range(HPC):
                p, o = h // 2, 64 * (h % 2)
                for qc in range(NSC):
                    nkc = 4 * (qc + 1)
                    ps_ao = pp.tile((128, SC), FP32, name="ps_ao", bufs=2)
                    p_prev = None
                    for kc in range(nkc):
                        ps_sc = pp.tile((128, SC), FP32, name="ps_sc", bufs=2)
                        nc.tensor.matmul(
                            ps_sc,
                            kr_s[o:o + 64, p, 128 * kc:128 * kc + 128],
                            qr_s[o:o + 64, p, SC * qc:SC * (qc + 1)],
                            start=True, stop=True)
                        p_t = pool.tile((128, SC), FP32R, name="p_t", bufs=3)
                        nc.scalar.activation(out=p_t, in_=ps_sc, func=AF.Exp,
                                             scale=0.125)
                        r = kc - 4 * qc
                        if r >= 0:
                            nc.vector.tensor_tensor(
                                out=p_t, in0=p_t,
                                in1=mw_s[:, 384 - 128 * r:896 - 128 * r],
                                op=ALU.mult)
                        if p_prev is not None:
                            pk, pt = p_prev
                            nc.tensor.matmul(ps_ao, vh1_s[:, pk, h, :], pt,
                                             start=(pk == 0), stop=False)
                        p_prev = (kc, p_t)
                    pk, pt = p_prev
                    nc.tensor.matmul(ps_ao, vh1_s[:, pk, h, :], pt,
                                     start=(pk == 0), stop=True)
                    rc_t = pool.tile((64, SC), FP32, name="rc_t", bufs=2)
                    nc.vector.reciprocal(rc_t, ps_ao[64:128, :])
                    nc.vector.tensor_tensor(
                        out=ao_s[o:o + 64, p, SC * qc:SC * (qc + 1)],
                        in0=ps_ao[0:64, :], in1=rc_t, op=ALU.mult)

            # ---- o_proj partial: [S, HID] = ao.T @ woT
            for sc in range(NKC):
                ot_t = pool.tile((128, HID), FP32, name="ot_t", bufs=2)
                for hc in range(2):
                    ps_o = pp.tile((128, SC), FP32, name="ps_p", bufs=2)
                    for p in range(2):
                        nc.tensor.matmul(
                            ps_o, ao_s[:, p, 128 * sc:128 * sc + 128],
                            wo_s[:, p, SC * hc:SC * (hc + 1)],
                            start=(p == 0), stop=(p == 1))
                    nc.scalar.copy(out=ot_t[:, SC * hc:SC * (hc + 1)],
                                   in_=ps_o)
                nc.sync.dma_start(out=out_d[128 * sc:128 * sc + 128, :],
                                  in_=ot_t)

    _legalize_single_wait(nc)
    return nc


def _host_tables():
    j = np.arange(0, HD, 2, dtype=np.float64)
    theta = BASE ** (-j / HD)                      # [32]
    pos = np.arange(S, dtype=np.float64)
    mt = np.outer(theta, pos)                      # [32, S]
    cosT = np.tile(np.repeat(np.cos(mt), 2, axis=0), (2, 1)).astype(np.float32)
    sinT = np.tile(np.repeat(np.sin(mt), 2, axis=0), (2, 1)).astype(np.float32)
    r2s = np.zeros((128, 128), np.float32)
    i = np.arange(0, 128, 2)
    r2s[i, i + 1] = 1.0    # (R2s.T @ x)[2i+1] = +x[2i]
    r2s[i + 1, i] = -1.0   # (R2s.T @ x)[2i]   = -x[2i+1]
    ii = np.arange(128)[:, None]
    tt = np.arange(896)[None, :]
    mw = (ii <= tt - 384).astype(np.float32)
    return cosT, sinT, r2s, mw


_CACHE = {}


def _get_runner():
    if "runner" in _CACHE:
        return _CACHE["runner"]
    import jax
    from jax.sharding import Mesh, PartitionSpec
    from jax.experimental.shard_map import shard_map
    from concourse import mybir
    import concourse.bass2jax as bass2jax

    nc = _build_nc()
    bass2jax.install_neuronx_cc_hook()

    partition_name = (nc.partition_id_tensor.name
                      if nc.partition_id_tensor else None)
    in_names, out_names, out_avals, zero_shapes = [], [], [], []
    for alloc in nc.m.functions[0].allocations:
        if not isinstance(alloc, mybir.MemoryLocationSet):
            continue
        name = alloc.memorylocations[0].name
        if alloc.kind == "ExternalInput":
            if name != partition_name:
                in_names.append(name)
        elif alloc.kind == "ExternalOutput":
            shape = tuple(alloc.tensor_shape)
            dtype = mybir.dt.np(alloc.dtype)
            out_avals.append(jax.core.ShapedArray(shape, dtype))
            out_names.append(name)
            zero_shapes.append((shape, dtype))
    n_params = len(in_names)
    all_in = list(in_names) + list(out_names)
    if partition_name is not None:
        all_in.append(partition_name)

    def _body(*args):
        operands = list(args)
        if partition_name is not None:
            operands.append(bass2jax.partition_id_tensor())
        outs = bass2jax._bass_exec_p.bind(
            *operands, out_avals=tuple(out_avals), in_names=tuple(all_in),
            out_names=tuple(out_names), lowering_input_output_aliases=(),
            sim_require_finite=True, sim_require_nnan=True, nc=nc)
        return tuple(outs)

    devices = jax.devices()[:NCORES]
    mesh = Mesh(np.asarray(devices), ("core",))
    in_specs = (PartitionSpec("core"),) * (n_params + len(out_names))
    out_specs = (PartitionSpec("core"),) * len(out_names)
    sharded = jax.jit(
        shard_map(_body, mesh=mesh, in_specs=in_specs, out_specs=out_specs,
                  check_rep=False),
        keep_unused=True)

    runner = {"sharded": sharded, "in_names": in_names, "mesh": mesh,
              "out_names": out_names, "zero_shapes": zero_shapes}
    _CACHE["runner"] = runner
    return runner


def _make_in_maps(q, k, v, Wq, Wk, Wv, Wo):
    cosT, sinT, r2s, mw = _host_tables()
    qT = [np.ascontiguousarray(q[b].T) for b in range(B)]
    kT = [np.ascontiguousarray(k[b].T) for b in range(B)]
    vT = [np.ascontiguousarray(v[b].T) for b in range(B)]
    WqT, WkT, WvT, WoT = Wq.T, Wk.T, Wv.T, Wo.T
    in_maps = []
    for c in range(NCORES):
        b, g = c // 4, c % 4
        in_maps.append({
            "xqT": qT[b], "xkT": kT[b], "xvT": vT[b],
            "wqT": np.ascontiguousarray(WqT[:, 256 * g:256 * (g + 1)]),
            "wkT": np.ascontiguousarray(WkT[:, 256 * g:256 * (g + 1)]),
            "wvT": np.ascontiguousarray(WvT[:, 256 * g:256 * (g + 1)]),
            "woT": np.ascontiguousarray(WoT[256 * g:256 * (g + 1), :]),
            "cosT": cosT, "sinT": sinT, "r2s": r2s, "maskw": mw,
        })
    return in_maps


def _run(in_maps):
    r = _get_runner()
    per_core = [[np.asarray(m[n]) for n in r["in_names"]] for m in in_maps]
    concat_in = [np.concatenate([per_core[c][i] for c in range(NCORES)],
                                axis=0) for i in range(len(r["in_names"]))]
    concat_zeros = [np.zeros((NCORES * s[0], *s[1:]), d)
                    for s, d in r["zero_shapes"]]
    outs = r["sharded"](*concat_in, *concat_zeros)
    return outs


def kernel(q, k, v, mask, Wq, Wk, Wv, Wo):
    in_maps = _make_in_maps(np.asarray(q), np.asarray(k), np.asarray(v),
                            np.asarray(Wq), np.asarray(Wk), np.asarray(Wv),
                            np.asarray(Wo))
    outs = _run(in_maps)
    part = np.asarray(outs[0]).reshape(NCORES, S, HID)
    out = np.empty((B, S, HID), np.float32)
    out[0] = part[0] + part[1] + part[2] + part[3]
    out[1] = part[4] + part[5] + part[6] + part[7]
    return out


# revision 14
# speedup vs baseline: 3848.1834x; 59.9523x over previous
"""Multi-head attention (B=2, S=2048, HID=1024, NH=16, HD=64) with
interleaved RoPE and causal softmax, sharded over 8 TRN2 NeuronCores:
data parallel on batch (cores 0-3 -> b=0, 4-7 -> b=1), tensor parallel
on heads (4 heads per core). Each core computes its heads' partial
o_proj [2048, 1024]; the host sums 4 partials per batch."""
import sys
sys.path.insert(0, "/opt/trn_rl_repo")
import numpy as np

B, S, HID, NH, HD = 2, 2048, 1024, 16, 64
BASE = 10000.0
NCORES = 8
HPC = 4           # heads per core
SC = 512          # free-dim chunk
NSC = S // SC     # 4
NKC = S // 128    # 16 k-chunks of 128
NIC = HID // 128  # 8 input-dim chunks


def _legalize_single_wait(nc):
    """This walrus build rejects instructions with >1 sync wait: split
    extra waits into single-wait Drain instructions on the same engine."""
    import bass_rust
    n_split = 0
    for _name, bbw in nc.bb_map.items():
        bb = getattr(bbw, "bb", bbw)
        out = []
        for inst in bb.instructions:
            si = inst.sync_info
            waits = list(si.on_wait) if si is not None and si.on_wait else []
            if len(waits) > 1:
                for w in waits[:-1]:
                    d = bass_rust.InstDrain(name=nc.get_next_instruction_name())
                    d.engine = inst.engine
                    d.sync_info = bass_rust.SyncInfo(on_wait=[w], on_update=[])
                    nc.register_instruction(d)
                    out.append(d)
                    n_split += 1
                inst.sync_info = bass_rust.SyncInfo(
                    on_wait=[waits[-1]],
                    on_update=list(si.on_update) if si.on_update else [],
                )
            out.append(inst)
        bb.instructions = out
    return n_split


def _build_nc(reps=1):
    import concourse.bass as bass
    import concourse.tile as tile
    from concourse import mybir

    FP32 = mybir.dt.float32
    FP32R = mybir.dt.float32r
    AF = mybir.ActivationFunctionType
    ALU = mybir.AluOpType

    nc = bass.Bass()
    xq_d = nc.dram_tensor("xqT", (HID, S), FP32R, kind="ExternalInput")
    xk_d = nc.dram_tensor("xkT", (HID, S), FP32R, kind="ExternalInput")
    xv_d = nc.dram_tensor("xvT", (HID, S), FP32R, kind="ExternalInput")
    wq_d = nc.dram_tensor("wqT", (HID, 256), FP32R, kind="ExternalInput")
    wk_d = nc.dram_tensor("wkT", (HID, 256), FP32R, kind="ExternalInput")
    wv_d = nc.dram_tensor("wvT", (HID, 256), FP32R, kind="ExternalInput")
    wo_d = nc.dram_tensor("woT", (256, HID), FP32R, kind="ExternalInput")
    cos_d = nc.dram_tensor("cosT", (128, S), FP32, kind="ExternalInput")
    sin_d = nc.dram_tensor("sinT", (128, S), FP32, kind="ExternalInput")
    r2s_d = nc.dram_tensor("r2s", (128, 128), FP32R, kind="ExternalInput")
    mw_d = nc.dram_tensor("maskw", (128, 896), FP32, kind="ExternalInput")
    out_d = nc.dram_tensor("partial", (S, HID), FP32, kind="ExternalOutput")

    def dma3(dst, src_t, row_stride, nchunk, ncol, offset):
        nc.sync.dma_start(out=dst, in_=bass.AP(
            tensor=src_t, offset=offset,
            ap=[[row_stride, 128], [128 * row_stride, nchunk], [1, ncol]]))

    with tile.TileContext(nc) as tc:
        with tc.tile_pool(name="sbuf", bufs=1) as pool, \
             tc.tile_pool(name="psum", bufs=1, space="PSUM") as pp:
            # persistent tiles
            wq_s = pool.tile((128, NIC, 256), FP32R)
            wk_s = pool.tile((128, NIC, 256), FP32R)
            wv_s = pool.tile((128, NIC, 256), FP32R)
            wo_s = pool.tile((128, 2, HID), FP32R)
            cos_s = pool.tile((128, S), FP32)
            sin_s = pool.tile((128, S), FP32)
            r2s_s = pool.tile((128, 128), FP32R)
            mw_s = pool.tile((128, 896), FP32)
            qr_s = pool.tile((128, 2, S), FP32R)
            kr_s = pool.tile((128, 2, S), FP32R)
            vh1_s = pool.tile((128, NKC, HPC, 128), FP32R)
            ao_s = pool.tile((128, 2, S), FP32R)

            for _rep in range(reps):
                dma3(wq_s, wq_d, 256, NIC, 256, 0)
                dma3(wk_s, wk_d, 256, NIC, 256, 0)
                dma3(wv_s, wv_d, 256, NIC, 256, 0)
                dma3(wo_s, wo_d, HID, 2, HID, 0)
                nc.sync.dma_start(out=cos_s, in_=cos_d[:, :])
                nc.sync.dma_start(out=sin_s, in_=sin_d[:, :])
                nc.sync.dma_start(out=r2s_s, in_=r2s_d[:, :])
                nc.sync.dma_start(out=mw_s, in_=mw_d[:, :])
                nc.vector.memset(vh1_s.bitcast(FP32)[:, :, :, 64:128], 1.0)

                # ---- q/k projections + RoPE -> qr_s/kr_s [128(pair), 2, S]
                for x_d, w_s, dst in ((xq_d, wq_s, qr_s), (xk_d, wk_s, kr_s)):
                    for c4 in range(NSC):
                        x_t = pool.tile((128, NIC, SC), FP32R, name="x_t",
                                        bufs=2)
                        dma3(x_t, x_d, S, NIC, SC, SC * c4)
                        for p in range(2):
                            ps_p = pp.tile((128, SC), FP32, name="ps_p",
                                           bufs=2)
                            for ck in range(NIC):
                                nc.tensor.matmul(
                                    ps_p, w_s[:, ck, 128 * p:128 * p + 128],
                                    x_t[:, ck, :],
                                    start=(ck == 0), stop=(ck == NIC - 1))
                            qh_t = pool.tile((128, SC), FP32R, name="qh_t",
                                             bufs=2)
                            nc.scalar.copy(out=qh_t, in_=ps_p)
                            ps_sw = pp.tile((128, SC), FP32, name="ps_sw",
                                            bufs=2)
                            nc.tensor.matmul(ps_sw, r2s_s, qh_t,
                                             start=True, stop=True)
                            dv = dst[:, p, SC * c4:SC * (c4 + 1)]
                            nc.vector.tensor_tensor(
                                out=dv, in0=qh_t,
                                in1=cos_s[:, SC * c4:SC * (c4 + 1)],
                                op=ALU.mult)
                            rt_t = pool.tile((128, SC), FP32, name="rt_t",
                                             bufs=2)
                            nc.vector.tensor_tensor(
                                out=rt_t, in0=ps_sw,
                                in1=sin_s[:, SC * c4:SC * (c4 + 1)],
                                op=ALU.mult)
                            nc.vector.tensor_tensor(
                                out=dv, in0=dv, in1=rt_t, op=ALU.add)

                # ---- v projection -> vh1_s [128(s), kc, h, 0:64] (+ones)
                for c4 in range(NSC):
                    x_t = pool.tile((128, NIC, SC), FP32R, name="x_t", bufs=2)
                    dma3(x_t, xv_d, S, NIC, SC, SC * c4)
                    for sub in range(4):
                        ps_v = pp.tile((128, SC), FP32, name="ps_p", bufs=2)
                        for ck in range(NIC):
                            nc.tensor.matmul(
                                ps_v[:, 0:256],
                                x_t[:, ck, 128 * sub:128 * sub + 128],
                                wv_s[:, ck, :],
                                start=(ck == 0), stop=(ck == NIC - 1))
                        nc.vector.tensor_copy(
                            out=vh1_s[:, 4 * c4 + sub, :, 0:64],
                            in_=ps_v[:, 0:256])

                # ---- attention per (head, q-chunk)
                for h in range(HPC):
                    p, o = h // 2, 64 * (h % 2)
                    for qc in range(NSC):
                        nkc = 4 * (qc + 1)
                        ps_ao = pp.tile((128, SC), FP32, name="ps_ao", bufs=2)
                        p_prev = None
                        for kc in range(nkc):
                            ps_sc = pp.tile((128, SC), FP32, name="ps_sc",
                                            bufs=2)
                            nc.tensor.matmul(
                                ps_sc,
                                kr_s[o:o + 64, p, 128 * kc:128 * kc + 128],
                                qr_s[o:o + 64, p, SC * qc:SC * (qc + 1)],
                                start=True, stop=True)
                            p_t = pool.tile((128, SC), FP32R, name="p_t",
                                            bufs=3)
                            nc.scalar.activation(out=p_t, in_=ps_sc,
                                                 func=AF.Exp, scale=0.125)
                            r = kc - 4 * qc
                            if r >= 0:
                                nc.vector.tensor_tensor(
                                    out=p_t, in0=p_t,
                                    in1=mw_s[:, 384 - 128 * r:896 - 128 * r],
                                    op=ALU.mult)
                            if p_prev is not None:
                                pk, pt = p_prev
                                nc.tensor.matmul(ps_ao, vh1_s[:, pk, h, :],
                                                 pt, start=(pk == 0),
                                                 stop=False)
                            p_prev = (kc, p_t)
                        pk, pt = p_prev
                        nc.tensor.matmul(ps_ao, vh1_s[:, pk, h, :], pt,
                                         start=(pk == 0), stop=True)
                        rc_t = pool.tile((64, SC), FP32, name="rc_t", bufs=2)
                        nc.vector.reciprocal(rc_t, ps_ao[64:128, :])
                        nc.vector.tensor_tensor(
                            out=ao_s[o:o + 64, p, SC * qc:SC * (qc + 1)],
                            in0=ps_ao[0:64, :], in1=rc_t, op=ALU.mult)

                # ---- o_proj partial: [S, HID] = ao.T @ woT
                for sc in range(NKC):
                    ot_t = pool.tile((128, HID), FP32, name="ot_t", bufs=2)
                    for hc in range(2):
                        ps_o = pp.tile((128, SC), FP32, name="ps_p", bufs=2)
                        for p in range(2):
                            nc.tensor.matmul(
                                ps_o, ao_s[:, p, 128 * sc:128 * sc + 128],
                                wo_s[:, p, SC * hc:SC * (hc + 1)],
                                start=(p == 0), stop=(p == 1))
                        nc.scalar.copy(out=ot_t[:, SC * hc:SC * (hc + 1)],
                                       in_=ps_o)
                    nc.sync.dma_start(out=out_d[128 * sc:128 * sc + 128, :],
                                      in_=ot_t)

    _legalize_single_wait(nc)
    return nc


def _host_tables():
    j = np.arange(0, HD, 2, dtype=np.float64)
    theta = BASE ** (-j / HD)                      # [32]
    pos = np.arange(S, dtype=np.float64)
    mt = np.outer(theta, pos)                      # [32, S]
    cosT = np.tile(np.repeat(np.cos(mt), 2, axis=0), (2, 1)).astype(np.float32)
    sinT = np.tile(np.repeat(np.sin(mt), 2, axis=0), (2, 1)).astype(np.float32)
    r2s = np.zeros((128, 128), np.float32)
    i = np.arange(0, 128, 2)
    r2s[i, i + 1] = 1.0    # (R2s.T @ x)[2i+1] = +x[2i]
    r2s[i + 1, i] = -1.0   # (R2s.T @ x)[2i]   = -x[2i+1]
    ii = np.arange(128)[:, None]
    tt = np.arange(896)[None, :]
    mw = (ii <= tt - 384).astype(np.float32)
    return cosT, sinT, r2s, mw


_CACHE = {}


def _get_runner(reps=1):
    key = "runner%d" % reps
    if key in _CACHE:
        return _CACHE[key]
    import jax
    from jax.sharding import Mesh, PartitionSpec
    from jax.experimental.shard_map import shard_map
    from concourse import mybir
    import concourse.bass2jax as bass2jax

    nc = _build_nc(reps)
    bass2jax.install_neuronx_cc_hook()

    partition_name = (nc.partition_id_tensor.name
                      if nc.partition_id_tensor else None)
    in_names, out_names, out_avals, zero_shapes = [], [], [], []
    for alloc in nc.m.functions[0].allocations:
        if not isinstance(alloc, mybir.MemoryLocationSet):
            continue
        name = alloc.memorylocations[0].name
        if alloc.kind == "ExternalInput":
            if name != partition_name:
                in_names.append(name)
        elif alloc.kind == "ExternalOutput":
            shape = tuple(alloc.tensor_shape)
            dtype = mybir.dt.np(alloc.dtype)
            out_avals.append(jax.core.ShapedArray(shape, dtype))
            out_names.append(name)
            zero_shapes.append((shape, dtype))
    n_params = len(in_names)
    all_in = list(in_names) + list(out_names)
    if partition_name is not None:
        all_in.append(partition_name)

    def _body(*args):
        operands = list(args)
        if partition_name is not None:
            operands.append(bass2jax.partition_id_tensor())
        outs = bass2jax._bass_exec_p.bind(
            *operands, out_avals=tuple(out_avals), in_names=tuple(all_in),
            out_names=tuple(out_names), lowering_input_output_aliases=(),
            sim_require_finite=True, sim_require_nnan=True, nc=nc)
        return tuple(outs)

    devices = jax.devices()[:NCORES]
    mesh = Mesh(np.asarray(devices), ("core",))
    in_specs = (PartitionSpec("core"),) * (n_params + len(out_names))
    out_specs = (PartitionSpec("core"),) * len(out_names)
    sharded = jax.jit(
        shard_map(_body, mesh=mesh, in_specs=in_specs, out_specs=out_specs,
                  check_rep=False),
        keep_unused=True)

    runner = {"sharded": sharded, "in_names": in_names, "mesh": mesh,
              "out_names": out_names, "zero_shapes": zero_shapes}
    _CACHE[key] = runner
    return runner


def _make_in_maps(q, k, v, Wq, Wk, Wv, Wo):
    cosT, sinT, r2s, mw = _host_tables()
    qT = [np.ascontiguousarray(q[b].T) for b in range(B)]
    kT = [np.ascontiguousarray(k[b].T) for b in range(B)]
    vT = [np.ascontiguousarray(v[b].T) for b in range(B)]
    WqT, WkT, WvT, WoT = Wq.T, Wk.T, Wv.T, Wo.T
    in_maps = []
    for c in range(NCORES):
        b, g = c // 4, c % 4
        in_maps.append({
            "xqT": qT[b], "xkT": kT[b], "xvT": vT[b],
            "wqT": np.ascontiguousarray(WqT[:, 256 * g:256 * (g + 1)]),
            "wkT": np.ascontiguousarray(WkT[:, 256 * g:256 * (g + 1)]),
            "wvT": np.ascontiguousarray(WvT[:, 256 * g:256 * (g + 1)]),
            "woT": np.ascontiguousarray(WoT[256 * g:256 * (g + 1), :]),
            "cosT": cosT, "sinT": sinT, "r2s": r2s, "maskw": mw,
        })
    return in_maps


def _run(in_maps):
    r = _get_runner()
    per_core = [[np.asarray(m[n]) for n in r["in_names"]] for m in in_maps]
    concat_in = [np.concatenate([per_core[c][i] for c in range(NCORES)],
                                axis=0) for i in range(len(r["in_names"]))]
    concat_zeros = [np.zeros((NCORES * s[0], *s[1:]), d)
                    for s, d in r["zero_shapes"]]
    outs = r["sharded"](*concat_in, *concat_zeros)
    return outs


def kernel(q, k, v, mask, Wq, Wk, Wv, Wo):
    in_maps = _make_in_maps(np.asarray(q), np.asarray(k), np.asarray(v),
                            np.asarray(Wq), np.asarray(Wk), np.asarray(Wv),
                            np.asarray(Wo))
    outs = _run(in_maps)
    part = np.asarray(outs[0]).reshape(NCORES, S, HID)
    out = np.empty((B, S, HID), np.float32)
    out[0] = part[0] + part[1] + part[2] + part[3]
    out[1] = part[4] + part[5] + part[6] + part[7]
    return out
